# revision 19
# baseline (speedup 1.0000x reference)
"""Contrastive loss (SimCLR-style) TRN2 Bass kernel, 8-core data-parallel.

Math: z [8192, 256] f32 ->
  zn = z / ||z||row ; S = (zn @ zn.T)/0.1 ; diag masked; row log_softmax;
  loss = -mean_i( S[i, pos(i)] - logsumexp_j S[i, j] ), pos(i) = (i+4096) % 8192.

Strategy (per sharding hint): shard rows across 8 cores (1024 rows each).
Each core normalizes the full z for the column operand (bf16, DMA-transposed
via a DRAM bounce), but keeps its OWN row block raw: the row-norm factor is
folded into the exp activation's per-partition scale AP (exp(s*10*rinv_i-10)),
so the lhsT transpose chain has no DVE dependency and matmuls start early.
The self term exp(10*d_ii-10) ~= 1 is subtracted on the host. The positive
logit comes from a raw elementwise dot with the partner block times both
rsqrt factors. ACT (exp drain at 1 elem/cycle/partition) is the roofline
(~55us/core); everything else is shaped to stay off its critical path:
1-Newton Quake rsqrt, f16 squares/reductions, group-pipelined producers.
"""

import numpy as np

N = 8192
K = 256
N_CORES = 8
BLK = N // N_CORES          # 1024 rows per core
MT = BLK // 128             # 8 m-tiles per core
NT = N // 128               # 64 row tiles of full z
GROUP_TILES = [4, 4, 8, 16, 16, 16]  # row tiles per pipeline group (sums to NT)
TEMP_INV = 10.0             # 1/temperature
QMAGIC = 0x5F3759DF

_CACHE = {}


def _build():
    import concourse.bass as bass
    import concourse.tile as tile
    from concourse import bacc, mybir
    from concourse.bass_interp import get_hw_module

    F32, BF16, F16 = mybir.dt.float32, mybir.dt.bfloat16, mybir.dt.float16
    I32 = mybir.dt.int32
    AF, ALU = mybir.ActivationFunctionType, mybir.AluOpType
    AX = mybir.AxisListType

    nc = bacc.Bacc("TRN2", target_bir_lowering=False, debug=False,
                   enable_asserts=False, num_devices=N_CORES)

    zf_in = nc.dram_tensor("zf", [N, K], F32, kind="ExternalInput").ap()
    zb_in = nc.dram_tensor("zb", [BLK, K], F32, kind="ExternalInput").ap()
    zp_in = nc.dram_tensor("zp", [BLK, K], F32, kind="ExternalInput").ap()
    dpos_out = nc.dram_tensor("dpos", [128, MT], F32, kind="ExternalOutput").ap()
    rs_out = nc.dram_tensor("rs", [128, MT], F32, kind="ExternalOutput").ap()

    with tile.TileContext(nc) as tc:
        with (
            tc.tile_pool(name="big", bufs=1) as big,
            tc.tile_pool(name="pipe", bufs=3) as pipe,
            tc.tile_pool(name="work", bufs=2) as work,
            tc.tile_pool(name="stat", bufs=1) as stat,
            tc.tile_pool(name="dram", bufs=1, space=bass.MemorySpace.DRAM) as dram,
            tc.tile_pool(name="ps", bufs=2, space=bass.MemorySpace.PSUM) as psp,
        ):
            magic = stat.tile([128, NT], I32)
            nc.vector.memset(magic[:], QMAGIC)
            bias_m10 = stat.tile([128, 1], F32)
            nc.vector.memset(bias_m10[:], -TEMP_INV)

            def rsqrt_dve(ss, nt, tag):
                """rsq = 1/sqrt(ss), Quake init + 1 Newton step (DVE only).

                Max rel err ~5e-6 — far below the bf16 rounding of zn."""
                ssi = ss[:].bitcast(I32)
                sh = work.tile([128, nt], I32, tag="sh")
                nc.vector.tensor_scalar(sh[:], ssi, 1, None,
                                        op0=ALU.arith_shift_right)
                y = stat.tile([128, nt], F32, tag=f"y_{tag}")
                yi = y[:].bitcast(I32)
                nc.vector.tensor_sub(yi, magic[:, 0:nt], sh[:])
                y2 = work.tile([128, nt], F32, tag="nwt")
                nc.vector.tensor_mul(y2[:], y[:], y[:])
                xy2 = work.tile([128, nt], F32, tag="nwt")
                nc.vector.tensor_mul(xy2[:], ss[:], y2[:])
                c = work.tile([128, nt], F32, tag="nwt")
                nc.vector.tensor_scalar(c[:], xy2[:], -0.5, 1.5,
                                        op0=ALU.mult, op1=ALU.add)
                yn = stat.tile([128, nt], F32, tag=f"yn_{tag}")
                nc.vector.tensor_mul(yn[:], y[:], c[:])
                return yn

            def row_ss(zt, nt, tag):
                """sum of squares per row of a [128, nt, K] bf16 tile.

                f16 reduce keeps the DVE 2x fast path; a tiny f16->f32 copy
                feeds the f32 Quake bit-trick."""
                sq = work.tile([128, nt, K], F16, tag="sq")
                nc.vector.tensor_mul(sq[:], zt[:], zt[:])
                ss = stat.tile([128, nt], F32, tag=f"ss_{tag}")
                nc.vector.reduce_sum(ss[:], sq[:], axis=AX.X)
                return ss

            # ---- loads (casting DMAs are SWDGE-only): g0 first, then own
            # block, then the remaining groups, partner last ----
            def load_group(tpg, c0):
                # one buffer per group: loads never wait on earlier groups
                zt = pipe.tile([128, tpg, K], BF16, tag="zbf",
                               bufs=len(GROUP_TILES), name="zt")
                nc.gpsimd.dma_start(
                    zt[:], zf_in[c0:c0 + tpg * 128, :].rearrange(
                        "(p t) k -> p t k", p=128))
                return zt

            g_zbf = {0: load_group(GROUP_TILES[0], 0)}
            zbf_b = big.tile([128, MT, K], BF16, tag="zbf_b")
            nc.gpsimd.dma_start(zbf_b[:], zb_in.rearrange("(p t) k -> p t k", p=128))

            # ---- lhsT from RAW rows: direct SBUF->SBUF per-tile transposes
            # (XBAR planar K-half mapping), no DRAM bounce, no DVE dep.
            # Emitted right after the zbf_b load: the tile scheduler assigns
            # cross-queue sem thresholds from emission order, so emitting
            # these late would make them wait on unrelated later loads. ----
            zbT = big.tile([128, 2, BLK], BF16, tag="zbT")
            for t in range(MT):
                nc.sync.dma_start(zbT[:, :, t * 128:(t + 1) * 128],
                                  zbf_b[:, t, :], transpose=True)

            _c0 = GROUP_TILES[0] * 128
            for g, tpg in enumerate(GROUP_TILES):
                if g > 0:
                    g_zbf[g] = load_group(tpg, _c0)
                    _c0 += tpg * 128
            zbf_p = big.tile([128, MT, K], BF16, tag="zbf_p")
            nc.gpsimd.dma_start(zbf_p[:], zp_in.rearrange("(p t) k -> p t k", p=128))

            def produce(g, tpg):
                """normalize group g, transpose each row tile straight out
                of SBUF into the planar [K-half, cols] matmul operand."""
                gw = tpg * 128
                ss_g = row_ss(g_zbf[g], tpg, f"g{g}")
                rsq_g = rsqrt_dve(ss_g, tpg, f"g{g}")
                zn = pipe.tile([128, tpg, K], BF16, tag="zn")
                znT = pipe.tile([128, 2, gw], BF16, tag="znT")
                for t in range(tpg):
                    nc.vector.tensor_scalar(zn[:, t, :], g_zbf[g][:, t, :],
                                            rsq_g[:, t:t + 1], None, op0=ALU.mult)
                    nc.sync.dma_start(znT[:, :, t * 128:(t + 1) * 128],
                                      zn[:, t, :], transpose=True)
                return znT

            # group 0's DVE chain first: it gates the first activation;
            # the own-row scale chain only has to beat act#1
            znT0 = produce(0, GROUP_TILES[0])
            ss_b = row_ss(zbf_b, MT, "b")
            rsq_b = rsqrt_dve(ss_b, MT, "b")
            scale_b = stat.tile([128, MT], F32)
            nc.vector.tensor_scalar(scale_b[:], rsq_b[:], TEMP_INV, None,
                                    op0=ALU.mult)

            # ---- main pipeline over column groups (small first for fast start) ----
            rs_part = stat.tile([128, MT, len(GROUP_TILES)], F32)
            for g, tpg in enumerate(GROUP_TILES):
                gw = tpg * 128
                znT = znT0 if g == 0 else produce(g, tpg)
                for mt in range(MT):
                    ps = psp.tile([128, gw], F32, tag="ps")
                    for sub in range(gw // 512):
                        psl = slice(sub * 512, (sub + 1) * 512)
                        nc.tensor.matmul(ps[:, psl],
                                         zbT[:, 0, mt * 128:(mt + 1) * 128],
                                         znT[:, 0, psl], start=True, stop=False)
                        nc.tensor.matmul(ps[:, psl],
                                         zbT[:, 1, mt * 128:(mt + 1) * 128],
                                         znT[:, 1, psl], start=False, stop=True)
                    # exp(s*10*rinv_i - 10) in place; only the row-sum survives
                    nc.scalar.activation(ps[:], ps[:], AF.Exp,
                                         bias=bias_m10[:],
                                         scale=scale_b[:, mt:mt + 1],
                                         accum_out=rs_part[:, mt, g:g + 1])

            # ---- tail: positive-pair dot from raw blocks (fills DVE gaps) ----
            ss_p = row_ss(zbf_p, MT, "p")
            rsq_p = rsqrt_dve(ss_p, MT, "p")
            dotbp = work.tile([128, MT, K], F16, tag="sq")
            nc.vector.tensor_mul(dotbp[:], zbf_b[:], zbf_p[:])
            dot = stat.tile([128, MT], F32)
            nc.vector.reduce_sum(dot[:], dotbp[:], axis=AX.X)
            dp1 = stat.tile([128, MT], F32)
            nc.vector.tensor_mul(dp1[:], dot[:], rsq_b[:])
            d_pos = stat.tile([128, MT], F32)
            nc.vector.tensor_mul(d_pos[:], dp1[:], rsq_p[:])

            rs_sum = stat.tile([128, MT], F32)
            nc.vector.reduce_sum(rs_sum[:], rs_part[:], axis=AX.X)

            nc.sync.dma_start(dpos_out, d_pos[:])
            nc.sync.dma_start(rs_out, rs_sum[:])

    nc.compile()
    nc.m = get_hw_module(nc.m)
    return nc


def _get_nc():
    if "nc" not in _CACHE:
        _CACHE["nc"] = _build()
    return _CACHE["nc"]


def _in_maps(z):
    z = np.ascontiguousarray(z, dtype=np.float32)
    maps = []
    for c in range(N_CORES):
        r0 = c * BLK
        p0 = (r0 + N // 2) % N
        maps.append({
            "zf": z,
            "zb": np.ascontiguousarray(z[r0:r0 + BLK]),
            "zp": np.ascontiguousarray(z[p0:p0 + BLK]),
        })
    return maps


def _finish(results):
    total = 0.0
    for c in range(N_CORES):
        dpos = results[c]["dpos"].astype(np.float64)
        rs = results[c]["rs"].astype(np.float64)
        # rowsum includes the self term exp(10*d_ii-10) ~= 1.0
        total += (TEMP_INV * dpos - TEMP_INV - np.log(rs - 1.0)).sum()
    return np.float32(-total / N)


def kernel(z):
    from concourse import bass_utils
    nc = _get_nc()
    res = bass_utils.run_bass_kernel_spmd(nc, _in_maps(z),
                                          core_ids=list(range(N_CORES)))
    return _finish(res.results)
